# revision 45
# baseline (speedup 1.0000x reference)
"""Trainium2 Bass kernel: 3x3 conv (stride 1, pad 1) + bias, NCHW.

x[16,128,112,112] * w[256,128,3,3] + b[256] -> y[16,256,112,112]

Data-parallel over batch (2 images per core, 8 cores). Per core, each
PSUM group (4 output rows x 112 cols, one 128-cout block) is one clean
9-matmul bf16 accumulation group. Inputs/weights are bf16 (rel-err
2.2e-3 vs the 2e-2 budget), PSUM accumulates fp32, bias is fused in the
ScalarE drain, output is stored as fp16 and upcast on host.

Streaming layout: each image is a flat padded plane [128cin, 13000]
(2 guard zeros + 114*114 + 2 spare). Every matmul streams 448 useful
columns as a strided [128, 4row x 114, 112col x 1] AP starting at
offset (r0+dy)*114+dx, so the tap shift is absorbed into the moving
operand's start offset and all 9 taps share one uniform contiguous
psum window [0,448) (strided streaming costs nothing on HW - measured;
an earlier contiguous-456 variant wasted 1.75%% on row-wrap junk cols).
Warmups are sized to bridge the head-DMA wait so the PE never idles
before the real stream (an idle gap resets the p-state ramp and the
first ~10 matmuls run at ~2x cost).

HW facts measured this session (TRN2, 456-col matmuls):
- bf16/fp8 matmul = stream + ~7 cyc (193ns); fp32r = 207-209ns
  (self-LDW bound) - hence bf16 over the old fp32r baseline.
- fp8-e4m3 DoubleRow = 193ns for 2 contraction planes (2x/plane), BUT:
  a DR matmul only works as a standalone start&stop group; stop=False
  or start=False on a DR wedges the device. Accumulating bf16 matmuls
  with start=False onto it costs a ~38ns RMW bubble each (8-matmul
  hybrid measured 229us vs 213us for clean bf16 groups), and the
  two-bank variant (bf16 group + standalone DR + DVE combine; DVE can
  read only ONE psum operand per op) wedged the device - unresolved.
  Plain-fp8 accuracy: 2 taps fp8 = 1.55e-2; 4 taps = 2.02e-2 (fails;
  1.66e-2 after offline error-feedback re-rounding of w8/x8 - see
  roundopt2.py / blob_*.py for that machinery).
- 4D ifmap APs ([128,2,4,112]) wedge the device.
- gpsimd SWDGE stores can't sustain the ~74GB/s output rate (adds
  ~11us tail); stores go on the sync+scalar HWDGE rings instead,
  which are idle once the input/weight loads finish.

Chunked input loads (first 10 rows, then 16-row chunks) on the sync
HWDGE ring, weights on the scalar ring; image-1 loads deferred into
image-0 compute; PE warmup matmuls span the head DMA window (p-state
ramp needs ~3us); last chunk split 8/4/4 for a short store tail.
"""
import numpy as np
import ml_dtypes
from concourse import bacc, mybir, bass
import concourse.tile as tile
from concourse.bass_utils import run_bass_kernel_spmd

F32R = mybir.dt.float32r
F32 = mybir.dt.float32
F16 = mybir.dt.float16
BF16 = mybir.dt.bfloat16
FP8 = mybir.dt.float8e4
DR = mybir.MatmulPerfMode.DoubleRow

B, CIN, H, W = 16, 128, 112, 112
COUT = 256
HP = WP = 114
PLANE = HP * WP          # 12996
GUARD = 2
XLEN = GUARD + PLANE + 2  # 13000
NCORES = 8
BPC = B // NCORES
NR = 4                   # output rows per PSUM group
NSTREAM = NR * W         # 448 moving cols per matmul (4 strided rows)
NCHUNK = 16              # output rows per staged out-chunk / store DMA
XCHUNKS = [(0, 15), (15, 31), (31, 47), (47, 63), (63, 79),
           (79, 95), (95, 111), (111, 114)]
NWARM = 36
# 7 bf16 taps; taps (0,0) and (0,1) ride the fp8 DoubleRow matmul.
# HW constraint (bisected): a DR matmul must be a standalone start&stop
# accumulation group - stop=False/start=False on a DR matmul wedges the
# device. stop_tensor_calc is a no-op on HW, so the DR matmul runs FIRST
# (start=True zeroes psum) and the bf16 taps accumulate on top with
# start=False + skip_group_check.
USE_FP8 = True
BF_TAPS = [(0, 2), (1, 0), (1, 1), (1, 2), (2, 0), (2, 1), (2, 2)]
F8_TAPS = [(0, 0), (0, 1)]
if not USE_FP8:
    BF_TAPS = F8_TAPS + BF_TAPS
    F8_TAPS = []
NBF = len(BF_TAPS)
# Packed weight layout (one head DMA, no round-robin dilution):
# bf16 cols [0,896)=cb0 taps | [896,1152)=w8 bytes | [1152,1156)=bias f32
# | [1156,2052)=cb1 taps.  Head transfer = cols [0,1156).
WCB = NBF * 128            # 896 bf16 cols per cout block
W8C = 256 if F8_TAPS else 0  # w8 as bf16-sized cols (2*2*128 fp8 bytes)
WBIAS = 4                  # bias as bf16 cols (2 f32)
WHEAD = WCB + W8C + WBIAS  # 1156
WTOT = WHEAD + WCB         # 2052

_cache = {}


def _build():
    nc = bacc.Bacc(None)
    xb_d = nc.dram_tensor("xb", [BPC, CIN, XLEN], BF16, kind="ExternalInput")
    x8_d = nc.dram_tensor("x8", [BPC, CIN, XLEN], FP8, kind="ExternalInput")
    wb_d = nc.dram_tensor("wtb", [CIN, WTOT], BF16, kind="ExternalInput")
    y_d = nc.dram_tensor("y", [BPC, COUT, H, W], F16, kind="ExternalOutput")

    with tile.TileContext(nc) as tc:
        with (
            tc.tile_pool(name="xbpool", bufs=BPC) as xbpool,
            tc.tile_pool(name="x8pool", bufs=BPC) as x8pool,
            tc.tile_pool(name="wbpool", bufs=1) as wbpool,
            tc.tile_pool(name="warms", bufs=1) as warms,
            tc.tile_pool(name="psA", bufs=4, space="PSUM") as psA,
            tc.tile_pool(name="psB", bufs=3, space="PSUM") as psB,
            tc.tile_pool(name="warmp", bufs=1, space="PSUM") as warmp,
            tc.tile_pool(name="tbpool", bufs=4) as tbpool,
            tc.tile_pool(name="opool", bufs=4) as opool,
        ):
            # PE warmup during the head DMA window (keeps the HAM activity
            # monitor from throttling the first real matmuls).
            wsrc = warms.tile([128, 128], F32)
            nc.vector.memset(wsrc[:], 0.0)
            wps = warmp.tile([128, 128], F32)
            for _ in range(NWARM):
                nc.tensor.matmul(wps[0:1, :], wsrc[:, 0:1].bitcast(F32R),
                                 wsrc[:].bitcast(F32R), start=True, stop=True)

            xb_t0 = xbpool.tile([CIN, XLEN], BF16, tag="xb")
            x8_t0 = x8pool.tile([CIN, XLEN], FP8, tag="x8")
            xb_ts, x8_ts = [xb_t0], [x8_t0]
            wbt = wbpool.tile([CIN, WTOT], BF16)
            w8v = wbt[:, WCB:WCB + W8C].bitcast(FP8) if F8_TAPS else None
            biasv = wbt[:, WCB + W8C:WHEAD].bitcast(F32)

            def w8_ap(cb):
                return bass.AP(tensor=w8v.tensor,
                               offset=w8v.offset + cb * 256,
                               ap=[w8v.ap[0], [128, 2], [1, 128]])

            def bias_ap(cb):
                return bass.AP(tensor=biasv.tensor,
                               offset=biasv.offset + cb,
                               ap=[biasv.ap[0], [1, 1]])

            def xload(img, c, eng=None, eng8=None):
                eng = eng or nc.sync
                a, b2 = XCHUNKS[c]
                s = 0 if a == 0 else GUARD + a * WP
                e = XLEN if b2 == HP else GUARD + b2 * WP
                eng.dma_start(xb_ts[img][:, s:e], xb_d[img, :, s:e])
                if F8_TAPS:
                    (eng8 or eng).dma_start(x8_ts[img][:, s:e],
                                            x8_d[img, :, s:e])

            # Head scheduling. Measured facts: each dma_start costs ~700ns
            # of descriptor-issue time on its engine, so big single
            # transfers beat fine-grained splits; the DMA queue
            # round-robins across queued transfers, so the cb=1 weight
            # half (needed only ~58us in) must NOT be queued at the head -
            # it steals bandwidth from w8t/wtb (measured +3us on stream
            # start); it is issued from the compute loop instead.
            # sync: x rows 0-14 (covers groups 0-2; the weight transfer
            # gates the stream start anyway), then 16-row chunks.
            # scalar: one packed transfer (cb0 taps + fp8 pair + bias).
            xload(0, 0)
            nc.scalar.dma_start(wbt[:, :WHEAD], wb_d[:, :WHEAD])
            for c in range(1, len(XCHUNKS)):
                xload(0, c)
            xb_t1 = xbpool.tile([CIN, XLEN], BF16, tag="xb")
            x8_t1 = x8pool.tile([CIN, XLEN], FP8, tag="x8")
            xb_ts.append(xb_t1)
            x8_ts.append(x8_t1)

            def emit_chunk(img, cb, c0, nrows, store_eng=None):
                xb_ap = xb_ts[img][:]
                x8_ap = x8_ts[img][:]
                ot = opool.tile([128, NCHUNK, W], F16, tag="o")
                for r0 in range(c0, c0 + nrows, NR):
                    # HW rules (bisected): a DR matmul must be a standalone
                    # start&stop accumulation group (stop=False wedges the
                    # device), and matmuls accumulating with start=False
                    # outside a proper group pay a ~38ns RMW bubble each.
                    # So: 7 bf16 taps as one clean group in bank A (192ns
                    # pacing), DR alone in bank B, combined in the drain.
                    # DVE can read only ONE psum operand per instruction, so
                    # the drain is: ScalarE activation drains B (+bias) to
                    # sbuf f32, then one DVE add of A(psum) + that -> fp16.
                    nbf = len(BF_TAPS)
                    # Contiguous 454-col stream (3*114 + 112; the last row
                    # needs no wrap junk): 2 junk cols per row wrap in
                    # psum, but no row-wrap AP bubbles on the PE (193ns/matmul
                    # measured vs 208ns for the strided [WP,NR],[1,W] form).
                    nst = (NR - 1) * WP + W
                    if F8_TAPS:
                        # fp8 DoubleRow pair in its own bank. HW rule
                        # (probed): the DR matmul must be issued BEFORE the
                        # bf16 group - a DR issued after a stop=True group
                        # wedges the device. Stream starts at the tap
                        # offset r0*114+2, which is even as DR requires.
                        psb = psB.tile([128, nst], F32, tag="b")
                        (dy0, dx0), (dy1, dx1) = F8_TAPS
                        off8 = GUARD + (r0 + dy0) * WP + dx0
                        delta = (dy1 - dy0) * WP + (dx1 - dx0)
                        mov8 = bass.AP(
                            tensor=x8_ap.tensor, offset=x8_ap.offset + off8,
                            ap=[x8_ap.ap[0], [delta, 2], [1, nst]])
                        nc.tensor.matmul(psb[:], w8_ap(cb), mov8,
                                         start=True, stop=True, perf_mode=DR)
                    psa = psA.tile([128, nst], F32, tag="a")
                    for ti, (dy, dx) in enumerate(BF_TAPS):
                        off = GUARD + (r0 + dy) * WP + dx
                        mov = bass.AP(
                            tensor=xb_ap.tensor, offset=xb_ap.offset + off,
                            ap=[xb_ap.ap[0], [1, nst]])
                        nc.tensor.matmul(
                            psa[:], wbt[:, cb * WHEAD + ti * 128:
                                        cb * WHEAD + (ti + 1) * 128],
                            mov, start=(ti == 0), stop=(ti == nbf - 1))
                    pr = psa[:]
                    a_rd = bass.AP(
                        tensor=pr.tensor, offset=pr.offset,
                        ap=[pr.ap[0], [WP, NR], [1, W]])
                    if F8_TAPS:
                        b_ap = psb[:]
                        b_rd = bass.AP(
                            tensor=b_ap.tensor, offset=b_ap.offset,
                            ap=[b_ap.ap[0], [WP, NR], [1, W]])
                        tmpb = tbpool.tile([128, NR, W], F32, tag="tb")
                        nc.scalar.activation(
                            tmpb[:], b_rd,
                            mybir.ActivationFunctionType.Identity,
                            bias=bias_ap(cb))
                        nc.vector.tensor_tensor(
                            ot[:, r0 - c0:r0 - c0 + NR, :], a_rd, tmpb[:],
                            mybir.AluOpType.add)
                    else:
                        # drain on DVE: psum read + per-partition bias +
                        # fp16 cast in one op; keeps scalar.activation (and
                        # its 1.4us ACT_TABLE_LOAD preamble) out of the
                        # kernel so the scalar ring's weight DMA starts
                        # earlier.
                        nc.vector.tensor_scalar_add(
                            ot[:, r0 - c0:r0 - c0 + NR, :], a_rd,
                            bias_ap(cb))
                (store_eng or nc.gpsimd).dma_start(
                    y_d[img, cb * 128:(cb + 1) * 128, c0:c0 + nrows, :],
                    ot[:, :nrows, :])

            # stores alternate between the sync and scalar HWDGE rings.
            # The gpsimd SWDGE ring generates descriptors in software and
            # cannot keep up with the ~74GB/s chunk rate (leaves ~11us of
            # end-of-kernel backlog); both HWDGE rings are essentially idle
            # once the input/weight loads finish. (vector cannot issue DMAs.)
            nchunk_i = [0]

            def store_ring():
                # NOTE: adding the gpsimd SWDGE ring to this rotation (even
                # only for early chunks) measured 250us vs 209us - its slow
                # software descriptor generation holds ot buffers (opool
                # bufs=4) long enough to stall the drain pipeline. Keep
                # stores strictly on the two fast HWDGE rings.
                nchunk_i[0] += 1
                return nc.sync if nchunk_i[0] % 2 else nc.scalar

            for img in range(BPC):
                for cb in range(2):
                    last = img == BPC - 1 and cb == 1
                    for ci, c0 in enumerate(range(0, H, NCHUNK)):
                        if last and c0 + NCHUNK >= H:
                            emit_chunk(img, cb, c0, 8, store_eng=store_ring())
                            emit_chunk(img, cb, c0 + 8, 4, store_eng=nc.sync)
                            emit_chunk(img, cb, c0 + 12, 4, store_eng=nc.scalar)
                        else:
                            emit_chunk(img, cb, c0, NCHUNK,
                                       store_eng=store_ring())
                        if img == 0 and cb == 0 and ci == 0:
                            # cb=1 weights, deferred past the head window
                            nc.scalar.dma_start(wbt[:, WHEAD:],
                                                wb_d[:, WHEAD:])
                        if img == 0 and cb == 0 and ci < 7:
                            # img1 x8 rides the scalar ring: the sync ring
                            # at ~208GB/s here is the slot-outlier source
                            xload(1, ci, eng8=nc.scalar)
                    if img == 0 and cb == 0:
                        for c in range(7, len(XCHUNKS)):
                            xload(1, c, eng8=nc.scalar)
    nc.compile()
    return nc


def _prep(x, weight, bias):
    x = np.asarray(x, dtype=np.float32)
    weight = np.asarray(weight, dtype=np.float32)
    bias = np.asarray(bias, dtype=np.float32)
    E4 = ml_dtypes.float8_e4m3
    BF = ml_dtypes.bfloat16

    xp = np.pad(x, ((0, 0), (0, 0), (1, 1), (1, 1)))
    flat = np.ascontiguousarray(xp.reshape(B, CIN, PLANE))
    xb = np.zeros((B, CIN, XLEN), dtype=BF)
    x8 = np.zeros((B, CIN, XLEN), dtype=E4)
    xb[:, :, GUARD:GUARD + PLANE] = flat.astype(BF)
    x8[:, :, GUARD:GUARD + PLANE] = flat.astype(E4)

    # Packed weight tensor: [0,WCB)=cb0 bf16 taps | [WCB,WCB+W8C)=fp8
    # pair weights (bytes) | bias (2 f32) | [WHEAD,WTOT)=cb1 bf16 taps.
    wbf = weight.astype(BF)
    wtb = np.zeros((CIN, WTOT), dtype=BF)
    for cb in range(2):
        for ti, (dy, dx) in enumerate(BF_TAPS):
            c0 = cb * WHEAD + ti * 128
            wtb[:, c0:c0 + 128] = wbf[cb * 128:(cb + 1) * 128, :, dy, dx].T
    ub = wtb.view(np.uint8)
    if F8_TAPS:
        w8f = weight.astype(E4)
        wt8 = np.zeros((CIN, 2, 2, 128), dtype=E4)
        for cb in range(2):
            for pi, (dy, dx) in enumerate(F8_TAPS):
                wt8[:, cb, pi] = w8f[cb * 128:(cb + 1) * 128, :, dy, dx].T
        ub[:, 2 * WCB:2 * WCB + 512] = wt8.view(np.uint8).reshape(CIN, 512)
    bt = np.ascontiguousarray(bias.reshape(2, 128).T.astype(np.float32))
    ub[:, 2 * (WCB + W8C):2 * (WCB + W8C) + 8] = bt.view(np.uint8)

    in_maps = [
        {
            "xb": np.ascontiguousarray(xb[c * BPC:(c + 1) * BPC]),
            "x8": np.ascontiguousarray(x8[c * BPC:(c + 1) * BPC]),
            "wtb": wtb,
        }
        for c in range(NCORES)
    ]
    return in_maps


def _run(x, weight, bias, **spmd_kwargs):
    if "nc" not in _cache:
        _cache["nc"] = _build()
    nc = _cache["nc"]
    in_maps = _prep(x, weight, bias)
    res = run_bass_kernel_spmd(nc, in_maps, list(range(NCORES)), **spmd_kwargs)
    y = np.concatenate(
        [np.asarray(res.results[c]["y"]) for c in range(NCORES)], axis=0)
    return y.astype(np.float32), res


def kernel(x, weight, bias):
    y, _ = _run(x, weight, bias)
    return y

